# revision 129
# baseline (speedup 1.0000x reference)
"""Bark-style causal self-attention on 8 Trainium2 NeuronCores.

Problem (hardcoded): B=4, S=1024, D=1024, H=16, hd=64, fp32 I/O.

Sharding: 8 cores = 4 batches x 2 head-groups (8 heads each).

v2: single fully-interleaved emission stream tuned against the
instruction-cost timeline model:
  - qk^T projection: 4 m-tiles swept k-major at boot (PE consumption rate
    matches the DMA arrival rate of the wqk/hsT chunks), remaining m-tiles
    interleaved into the attention pairs.
  - scores transposed as in v1 (pair-packed, 256-wide query chunks so a
    score tile fits one PSUM bank), exp on Activation, causal mask on DVE.
  - PV with p^T *stationary* and V moving (65 rows per matmul instead of
    ~128-512): ctx comes out natural [q, hd] with the softmax denominator
    in column 64; normalization is then a per-partition scalar multiply.
  - ctx^T recovered with PE transpose instructions (free Ldweights +
    128-row transposes), unloaded PSUM->SBUF on GpSimd.
  - out^T projection per (d, n) group with PSUM accumulation over the 4
    head pairs, n=0 half interleaved into pair 3, biases on GpSimd,
    output stored bf16 (host combines the two cores of a batch in fp32).
"""

from contextlib import ExitStack

import numpy as np
import ml_dtypes

import concourse.bass as bass
import concourse.tile as tile
import concourse.mybir as mybir
from concourse.bass_utils import run_bass_kernel_spmd
from concourse.vector_clock import ScopedClock


# --------------------------------------------------------------------------
# Workaround for the walrus build in this container, which accepts at most
# ONE sync-wait command per instruction (two on EventSemaphore).  Stock Tile
# emits instructions with several waits; we legalize the program after
# TileContext exit (see v1 for details).
# --------------------------------------------------------------------------

def _patched_drain_and_barrier(self, tick_clock, wait_clock):
    drain_inst = self.nc.sync.drain()
    wait_clock.add_sem_waits(
        drain_inst.ins, ScopedClock({None: tick_clock.global_clock})
    )
    si = drain_inst.ins.sync_info
    waits = list(si.on_wait or []) if si is not None else []
    if len(waits) > 1:
        si.on_wait = [waits[0]]
        for w in waits[1:]:
            extra = self.nc.sync.drain()
            esi = extra.ins.sync_info
            if esi is None:
                extra.ins.sync_info = mybir.SyncInfo(on_wait=[w], on_update=[])
            else:
                esi.on_wait = [w]

    self.nc.all_engine_barrier()
    assert self.sems is not None
    popped = self.nc._tile_sem_poison_stack.pop()
    assert popped is self._sem_poison
    self.nc.clear_and_free_semaphores(list(self.sems.allocated().values()))
    self.nc.all_engine_barrier()


tile.TileContext._drain_and_barrier = _patched_drain_and_barrier


def _legalize_waits_json(raw: bytes) -> bytes:
    """Split multi-wait instructions by inserting single-wait NoOp carriers
    immediately before them on the same engine."""
    import orjson

    j = orjson.loads(raw)
    for f in j["functions"]:
        for b in f["blocks"]:
            out = []
            for inst in b["instructions"]:
                si = inst.get("sync_info") or {}
                waits = si.get("on_wait") or []
                cap = 2 if inst.get("opcode") == "EventSemaphore" else 1
                if len(waits) > cap:
                    excess, keep = waits[:-cap], waits[-cap:]
                    for k, w in enumerate(excess):
                        out.append({
                            "debug": inst.get("debug", 0),
                            "engine": inst["engine"],
                            "ins": [],
                            "name": f"{inst['name']}-lw{k}",
                            "opcode": "NoOp",
                            "outs": [],
                            "sync_info": {"on_wait": [w]},
                        })
                    si["on_wait"] = keep
                    inst["sync_info"] = si
                out.append(inst)
            b["instructions"] = out
    return orjson.dumps(j)


BF16 = mybir.dt.bfloat16
F32 = mybir.dt.float32
NPBF16 = ml_dtypes.bfloat16

B, S, D, H, HD = 4, 1024, 1024, 16, 64
NCORES = 8
HPC = 8          # heads per core
PAIRS = 4        # head pairs per core
KCH = 8          # 128-row chunks of the D contraction
SCALE = 1.0 / np.sqrt(HD)
SCH = 256        # score chunk width (query dim); one PSUM bank per sT tile

TRACE = False
LAST_RESULTS = None

_CACHE = {}
DEBUG_DUMP = False


def _chunks(lo, hi, step):
    out = []
    while lo < hi:
        nxt = min(hi, (lo // step + 1) * step)
        out.append((lo, nxt))
        lo = nxt
    return out


def _emit(tc, io, ctx):
    nc = tc.nc
    hsT, wqk, qkb, wv, wout, tri, outT = (
        io["hsT"], io["wqk"], io["qkb"], io["wv"], io["wout"],
        io["tri"], io["outT"],
    )
    Exp = mybir.ActivationFunctionType.Exp

    persist = ctx.enter_context(tc.tile_pool(name="persist", bufs=1))

    def ptile(name, shape, dtype=BF16):
        return persist.tile(shape, dtype, name=name, tag=name)

    # ---- persistent SBUF tensors ----------------------------------------
    qkb_sb = ptile("qkb", [128, 8], F32)
    wqk_sb = [ptile(f"wqk{k}", [128, 1024]) for k in range(KCH)]
    hsT_sb = [ptile(f"hsT{k}", [128, S]) for k in range(KCH)]
    tri_sb = ptile("tri", [128, 128])
    wv_sb = [ptile(f"wv{k}", [128, 512]) for k in range(KCH)]
    wout_sb = [ptile(f"wout{p}", [128, 1024]) for p in range(PAIRS)]

    qkT_sb = [ptile(f"qkT{m}", [128, S]) for m in range(8)]
    v_sb = [ptile(f"v{s}", [128, HPC, 65]) for s in range(8)]
    ctxT_sb = [ptile(f"ctxT{p}", [128, S]) for p in range(PAIRS)]
    ctn_sb = [ptile(f"ctn{p}", [128, 8, 2, HD]) for p in range(PAIRS)]

    # ---- DMA loads (SP queue, in order of first use) --------------------
    # wqk[0][:, 0:768] covers the m in {0, 1, 4, 5} column slices the boot
    # sweep needs; the first matmul can start after just 2 transfers.
    nc.sync.dma_start(out=wqk_sb[0][:, 0:768], in_=wqk[0:128, 0:768])
    nc.sync.dma_start(out=hsT_sb[0][:, 0:512], in_=hsT[0:128, 0:512])
    nc.sync.dma_start(out=hsT_sb[0][:, 512:1024], in_=hsT[0:128, 512:1024])
    for k in range(1, KCH):
        r = slice(k * 128, (k + 1) * 128)
        nc.sync.dma_start(out=wqk_sb[k][:, 0:768], in_=wqk[r, 0:768])
        nc.sync.dma_start(out=hsT_sb[k][:, :], in_=hsT[r, :])
    nc.sync.dma_start(out=qkb_sb[:, :], in_=qkb[:, :])
    nc.sync.dma_start(out=tri_sb[:, :], in_=tri[:, :])
    for k in range(KCH):
        nc.sync.dma_start(out=wv_sb[k][:, :], in_=wv[k * 128:(k + 1) * 128, :])
    for k in range(KCH):   # m in {6, 7} slices, first used in pair 1
        nc.sync.dma_start(out=wqk_sb[k][:, 768:1024],
                          in_=wqk[k * 128:(k + 1) * 128, 768:1024])
    for p in range(PAIRS):
        nc.sync.dma_start(out=wout_sb[p][:, :],
                          in_=wout[p * 128:(p + 1) * 128, :])

    # ---- pools ----------------------------------------------------------
    # PSUM budget: boot(6) + pj(2) = 8 early; pj(2)+sT(2)+ctx(3)+T(1) = 8
    # once boot closes.
    pj_pool = ctx.enter_context(tc.tile_pool(name="pj", bufs=2, space="PSUM"))
    # SBUF working pools
    pt_pool = ctx.enter_context(tc.tile_pool(name="pt", bufs=14))
    rc_pool = ctx.enter_context(tc.tile_pool(name="rc", bufs=2))
    osb_pool = ctx.enter_context(tc.tile_pool(name="osb", bufs=8))

    # ---------------------------------------------------------------------
    # emission helpers
    # ---------------------------------------------------------------------
    def qk_bias(m, ps_n, act_n0=False):
        """PSUM -> SBUF with per-feature bias; the n=1 half (and optionally
        the n=0 half) unloads via an Act copy (+ in-place DVE add) so the
        boot handoff isn't serialized on DVE alone."""
        if act_n0:
            nc.scalar.copy(qkT_sb[m][:, 0:512], ps_n[0][:, :])
            nc.vector.tensor_scalar_add(
                qkT_sb[m][:, 0:512], qkT_sb[m][:, 0:512],
                qkb_sb[:, m:m + 1])
        else:
            nc.vector.tensor_scalar_add(
                qkT_sb[m][:, 0:512], ps_n[0][:, :], qkb_sb[:, m:m + 1])
        nc.scalar.copy(qkT_sb[m][:, 512:1024], ps_n[1][:, :])
        nc.vector.tensor_scalar_add(
            qkT_sb[m][:, 512:1024], qkT_sb[m][:, 512:1024],
            qkb_sb[:, m:m + 1])

    def proj_sweep_pieces(m):
        """k-sweep for one qk m-tile as 9 small pieces (for interleaving)."""
        ps = [None, None]

        def piece(k):
            if k == 0:
                for n in range(2):
                    ps[n] = pj_pool.tile([128, 512], F32,
                                         name=f"pj{m}_{n}", tag="pj")
            for n in range(2):
                nc.tensor.matmul(
                    ps[n][:, :],
                    lhsT=wqk_sb[k][:, m * 128:(m + 1) * 128],
                    rhs=hsT_sb[k][:, n * 512:(n + 1) * 512],
                    start=(k == 0), stop=(k == KCH - 1))

        for k in range(KCH):
            yield lambda k=k: piece(k)
        yield lambda: qk_bias(m, ps)

    def v_proj(s):
        """V projection chunk s: psum -> v_sb[s] (copy on DVE) + ones col."""
        ps = pj_pool.tile([128, 512], F32, name=f"vps{s}", tag="pj")
        for k in range(KCH):
            nc.tensor.matmul(
                ps[:, :],
                lhsT=hsT_sb[k][:, s * 128:(s + 1) * 128],
                rhs=wv_sb[k][:, :],
                start=(k == 0), stop=(k == KCH - 1))
        nc.vector.tensor_copy(v_sb[s][:, :, 0:64],
                              ps.rearrange("p (h c) -> p h c", c=64))
        nc.vector.memset(v_sb[s][:, :, 64:65], 1.0)

    # per-pair attention state
    def scores(p, kb):
        """Pair-packed transposed score chunks + exp + mask (v1 pattern:
        each matmul output fills its own PSUM bank)."""
        q0 = kb * 128
        for (c0, c1) in _chunks(0, S - q0, 512):
            wc = c1 - c0
            sT = sT_pool.tile([128, 2, 512], F32, name=f"sT{p}_{kb}_{c0}",
                              tag="sT")
            for t in range(2):
                nc.tensor.matmul(
                    sT[:, t, 0:wc],
                    lhsT=qkT_sb[4 + p][64 * t:64 * t + 64, q0:q0 + 128],
                    rhs=qkT_sb[p][64 * t:64 * t + 64, q0 + c0:q0 + c1],
                    start=True, stop=True,
                    tile_position=(64 * t, 0))
            pt = pt_pool.tile([128, 2, 512], BF16, name=f"pT{p}_{kb}_{c0}",
                              tag="pT")
            nc.scalar.activation(pt[:, :, 0:wc], sT[:, :, 0:wc], Exp,
                                 scale=SCALE)
            if c0 == 0:
                # causal mask on the diagonal 128x128 block, both heads
                pm = pt[:, :, 0:128]
                tri3 = tri_sb.rearrange("p (o c) -> p o c", o=1)
                tri_b, _ = bass.broadcast_tensor_aps(tri3, pm)
                nc.vector.tensor_mul(pm, pm, tri_b)
            yield pt, c0, c1

    def pv_qb(p, qb, pts):
        """p-stationary PV for one query block, both heads: ctx comes out
        natural [q, 65] (65 moving rows per matmul), the softmax denominator
        is per-partition (cheap normalize), and ctx^T is recovered with a
        hardware DMA transpose.  One accumulation group per PSUM bank."""
        for t in range(2):
            ct = ctx_pool.tile([128, 65], F32, name=f"cx{p}_{qb}_{t}",
                               tag="ctx")
            for kb in range(qb + 1):
                off = (qb - kb) * 128
                pt, c0, c1 = pts[kb][off // 512]
                sl = off - c0
                nc.tensor.matmul(
                    ct[:, :],
                    lhsT=pt[:, t, sl:sl + 128],
                    rhs=v_sb[kb][:, 2 * p + t, :],
                    start=(kb == 0), stop=(kb == qb))
            rc = rc_pool.tile([128, 1], F32, name=f"rc{p}{qb}{t}", tag="rc")
            nc.vector.reciprocal(rc[:, :], ct[:, 64:65])
            nc.vector.tensor_scalar_mul(ctn_sb[p][:, qb, t, :],
                                        ct[:, 0:64], rc[:, 0:1])
        nc.sync.dma_start_transpose(
            ctxT_sb[p][:, qb * 128:(qb + 1) * 128], ctn_sb[p][:, qb, :, :])

    ph4_state = {}

    def ph4_mm(ps, d, n, p, cols=None):
        c0, c1 = cols if cols is not None else (n * 512, (n + 1) * 512)
        nc.tensor.matmul(
            ps[:, c0 - n * 512:c1 - n * 512],
            lhsT=wout_sb[p][:, d * 128:(d + 1) * 128],
            rhs=ctxT_sb[p][:, c0:c1],
            start=(p == 0), stop=(p == PAIRS - 1),
            skip_group_check=cols is not None)

    def phase4_head(d, n, pool=None):
        """Pairs 0..2 of out^T tile (d, n) (not gated on pair 3)."""
        pool = pool if pool is not None else pj_pool
        ps = pool.tile([128, 512], F32, name=f"o{d}_{n}", tag="pj")
        ph4_state[(d, n)] = ps
        for p in range(3):
            ph4_mm(ps, d, n, p)

    osb_tiles = {}

    def phase4_tail(d, n, on_dve=False):
        """Pair-3 matmul + bf16 unload (the output bias is added on the
        host).  Both n-halves collect into one osb tile; a single combined
        DMA per d fires with the n=1 half (8 stores instead of 16)."""
        ps = ph4_state.pop((d, n))
        ph4_mm(ps, d, n, 3)
        if d not in osb_tiles:
            osb_tiles[d] = osb_pool.tile([128, 1024], BF16, name=f"ob{d}",
                                         tag="osb")
        osb = osb_tiles[d]
        if on_dve:
            nc.vector.tensor_copy(osb[:, n * 512:(n + 1) * 512], ps[:, :])
        else:
            nc.scalar.copy(osb[:, n * 512:(n + 1) * 512], ps[:, :])
        # d 5-7 finish last: fire their n=0 halves early (HWDGE is idle
        # then) so only half-sized transfers remain on the critical tail
        if d >= 5:
            nc.sync.dma_start(
                out=outT[d * 128:(d + 1) * 128, n * 512:(n + 1) * 512],
                in_=osb[:, n * 512:(n + 1) * 512])
        elif n == 1:
            nc.sync.dma_start(out=outT[d * 128:(d + 1) * 128, :],
                              in_=osb[:, :])

    def phase4_group(d, n, on_dve=False):
        phase4_head(d, n)
        phase4_tail(d, n, on_dve=on_dve)

    # ---------------------------------------------------------------------
    # boot: m-tiles {0, 4, 1, 5} swept k-major, paced by the input DMAs
    # ---------------------------------------------------------------------
    boot_pool = tc.alloc_tile_pool(name="boot", bufs=1, space="PSUM")
    boot_ms = [0, 4, 1]      # tiles in boot pool (6 banks)
    pjm = 5                  # fourth tile in pj pool (2 banks)
    boot_ps = {m: [boot_pool.tile([128, 512], F32, name=f"bt{m}_{n}",
                                  tag=f"bt{m}_{n}")
                   for n in range(2)] for m in boot_ms}
    pj_ps = {pjm: [pj_pool.tile([128, 512], F32, name=f"pj5_{n}", tag="pj")
                   for n in range(2)]}
    for k in range(KCH):
        for n in range(2):
            for m in boot_ms + [pjm]:
                ps = boot_ps[m][n] if m in boot_ps else pj_ps[m][n]
                nc.tensor.matmul(
                    ps[:, :],
                    lhsT=wqk_sb[k][:, m * 128:(m + 1) * 128],
                    rhs=hsT_sb[k][:, n * 512:(n + 1) * 512],
                    start=(k == 0), stop=(k == KCH - 1))
    # bias order: m0/m4 unblock the pair-0 scores, m1 completes the boot
    # pool's readers (releases its banks to the attention pools), m5 frees
    # the two pj slots the V projection uses.
    qk_bias(0, boot_ps[0])
    qk_bias(4, boot_ps[4])
    qk_bias(1, boot_ps[1], act_n0=True)
    qk_bias(pjm, pj_ps[pjm])
    boot_pool.release()

    # attention pools (open after boot closes): ctx 2 + sT 2x2 + pj 2 = 8.
    # ctx is allocated first so sT (stack top) can be released right after
    # the last scores, freeing banks for pair 3's second ctx pool.
    ctx_pool = tc.alloc_tile_pool(name="ctxp", bufs=2, space="PSUM")
    sT_pool = tc.alloc_tile_pool(name="sT", bufs=2, space="PSUM")

    # ---------------------------------------------------------------------
    # attention pairs with interleaved projection / phase-4 work
    # ---------------------------------------------------------------------
    # Filler PE work queues, one per pair, consumed between the score and
    # PV blocks of each key block (that window is where PE would otherwise
    # stall on the exp -> mask chain).
    fillers = {
        0: [],                                  # pair 0 is filled by V proj
        1: list(proj_sweep_pieces(2)) + list(proj_sweep_pieces(6)),
        2: list(proj_sweep_pieces(3)) + list(proj_sweep_pieces(7)),
        # pair 3: pre-stage the first two phase-4 heads (pairs 0-2 only,
        # not gated on pair 3's ctx^T).
        3: [lambda: phase4_head(0, 0), lambda: phase4_head(1, 0)],
    }

    all_pts = {}

    def emit_scores(p, kb_lo=0):
        """Score/exp stream for a pair, with that pair's filler pieces."""
        fq = fillers[p]
        npiece = ([3, 3, 3, 2, 2, 2, 2, 1] if p != 3
                  else [0, 0, 1, 1] + [0] * 4)
        all_pts.setdefault(p, {})
        for kb in range(kb_lo, KCH):
            all_pts[p][kb] = list(scores(p, kb))
            if p == 0:
                v_proj(kb)
            else:
                for _ in range(npiece[kb]):
                    if fq:
                        fq.pop(0)()
        while fq:
            fq.pop(0)()

    for p in range(3):
        emit_scores(p, kb_lo=(0 if p == 0 else 1))
        for qb in range(KCH):
            pv_qb(p, qb, all_pts[p])
            if qb == 5:
                # pre-emit the next pair's first key block so Act starts
                # its exp stream before this pair's PV drains
                all_pts[p + 1] = {0: list(scores(p + 1, 0))}
        del all_pts[p]

    # pair 3: PV query blocks interleave between its two score batches
    # (kb 0 was pre-emitted during pair 2's PV)
    fq3 = fillers[3]
    for kb in range(1, 4):
        all_pts[3][kb] = list(scores(3, kb))
        if kb >= 1 and fq3:
            fq3.pop(0)()
    for qb in range(4):
        pv_qb(3, qb, all_pts[3])
    for kb in range(4, KCH):
        all_pts[3][kb] = list(scores(3, kb))
    # sT's 4 banks free here and ctx only needs 2: the 4-slot phase-4 pool
    # opens now so four more (ungated) heads fill pair-3's exp-cadence
    # stalls without losing end-stagger depth
    sT_pool.release()
    ph4b_pool = tc.alloc_tile_pool(name="ph4b", bufs=4, space="PSUM")
    order = ([(d, 0) for d in range(4)]
             + [(0, 1), (4, 0), (1, 1), (5, 0), (2, 1), (6, 0), (3, 1),
                (7, 0), (4, 1), (5, 1), (6, 1), (7, 1)])
    for qb in range(4, KCH):
        pv_qb(3, qb, all_pts[3])
        if qb >= 4 and qb - 4 < 4:
            phase4_head(*order[2 + (qb - 4)], pool=ph4b_pool)

    # ---------------------------------------------------------------------
    # phase 4: staggered (d, n) tails; each new head takes the slot its
    # just-emitted tail freed (same pool), keeping the in-order PE stream
    # deadlock-free.  n=0 tails first (only need ctx^T columns 0:512).
    # ---------------------------------------------------------------------
    head_pool = {(0, 0): pj_pool, (1, 0): pj_pool,
                 (2, 0): ph4b_pool, (3, 0): ph4b_pool,
                 (0, 1): ph4b_pool, (4, 0): ph4b_pool}
    for i, (d, n) in enumerate(order):
        phase4_tail(d, n, on_dve=i % 2 == 1)
        if i + 6 < len(order):
            head_pool[order[i + 6]] = head_pool[(d, n)]
            phase4_head(*order[i + 6], pool=head_pool[order[i + 6]])
    ph4b_pool.release()
    ctx_pool.release()

    if DEBUG_DUMP:
        for m in range(8):
            nc.sync.dma_start(out=io["dbg_qkT"][m * 128:(m + 1) * 128, :],
                              in_=qkT_sb[m][:, :])
        for s in range(8):
            nc.sync.dma_start(
                out=io["dbg_v"][s * 128:(s + 1) * 128, :],
                in_=v_sb[s].rearrange("p h c -> p (h c)"))
        for p in range(PAIRS):
            nc.sync.dma_start(out=io["dbg_ctxT"][p * 128:(p + 1) * 128, :],
                              in_=ctxT_sb[p][:, :])


def _build():
    nc = bass.Bass("TRN2", target_bir_lowering=False, debug=False,
                   num_devices=NCORES)
    io = {
        "hsT": nc.dram_tensor("hsT", [1024, S], BF16,
                              kind="ExternalInput").ap(),
        "wqk": nc.dram_tensor("wqk", [1024, 1024], BF16,
                              kind="ExternalInput").ap(),
        "qkb": nc.dram_tensor("qkb", [128, 8], F32,
                              kind="ExternalInput").ap(),
        "wv": nc.dram_tensor("wv", [1024, 512], BF16,
                             kind="ExternalInput").ap(),
        "wout": nc.dram_tensor("wout", [512, 1024], BF16,
                               kind="ExternalInput").ap(),
        "tri": nc.dram_tensor("tri", [128, 128], BF16,
                              kind="ExternalInput").ap(),
        "outT": nc.dram_tensor("outT", [1024, S], BF16,
                               kind="ExternalOutput").ap(),
    }
    if DEBUG_DUMP:
        io["dbg_qkT"] = nc.dram_tensor("dbg_qkT", [1024, S], BF16,
                                       kind="ExternalOutput").ap()
        io["dbg_v"] = nc.dram_tensor("dbg_v", [1024, HPC * 65], BF16,
                                     kind="ExternalOutput").ap()
        io["dbg_ctxT"] = nc.dram_tensor("dbg_ctxT", [512, S], BF16,
                                        kind="ExternalOutput").ap()
    with tile.TileContext(nc) as tc:
        with ExitStack() as ctx:
            _emit(tc, io, ctx)
    fixed = _legalize_waits_json(nc.to_json_bytes())
    nc.to_json_bytes = (lambda fixed=fixed: fixed)
    return nc


def _get_nc():
    if "nc" not in _CACHE:
        _CACHE["nc"] = _build()
    return _CACHE["nc"]


def _prep_inputs(hidden_states, att_w, att_b, out_w, out_b):
    """Build the 8 per-core input maps (host-side shard/layout prep)."""
    hs = np.asarray(hidden_states, dtype=np.float32)
    att_w = np.asarray(att_w, dtype=np.float32)
    att_b = np.asarray(att_b, dtype=np.float32)
    out_w = np.asarray(out_w, dtype=np.float32)
    out_b = np.asarray(out_b, dtype=np.float32)

    tri = np.triu(np.ones((128, 128), dtype=np.float32)).astype(NPBF16)

    hsT_all = [np.ascontiguousarray(hs[b].T.astype(NPBF16))
               for b in range(B)]
    per_hg = []
    for hg in range(2):
        lo, hi = hg * 512, (hg + 1) * 512
        wqk = np.ascontiguousarray(
            np.concatenate([att_w[:, lo:hi], att_w[:, D + lo:D + hi]],
                           axis=1).astype(NPBF16))
        qkb = np.concatenate([att_b[lo:hi], att_b[D + lo:D + hi]])
        qkb = np.ascontiguousarray(qkb.reshape(8, 128).T).astype(np.float32)
        wv = np.ascontiguousarray(
            att_w[:, 2 * D + lo:2 * D + hi].astype(NPBF16))
        wout = np.ascontiguousarray(out_w[lo:hi, :].astype(NPBF16))
        per_hg.append((wqk, qkb, wv, wout))
    # Output bias applied on the host.  The v-bias passes through softmax
    # as a constant (weights sum to 1): ctx = ctx0 + bv, so bv @ w_out is
    # folded in here as well.
    host_bias = out_b + att_b[2 * D:3 * D] @ out_w
    in_maps = []
    for c in range(NCORES):
        b, hg = divmod(c, 2)
        wqk, qkb, wv, wout = per_hg[hg]
        in_maps.append({
            "hsT": hsT_all[b],
            "wqk": wqk,
            "qkb": qkb,
            "wv": wv,
            "wout": wout,
            "tri": tri,
        })
    return in_maps, host_bias


def kernel(hidden_states, att_w, att_b, out_w, out_b):
    global LAST_RESULTS
    in_maps, host_bias = _prep_inputs(hidden_states, att_w, att_b,
                                      out_w, out_b)
    nc = _get_nc()
    trace = TRACE
    if trace:
        try:
            from antenv.axon_hooks import get_axon_ntff_profile_hook  # noqa
        except ImportError:
            trace = False
    res = run_bass_kernel_spmd(nc, in_maps, core_ids=list(range(NCORES)),
                               trace=trace)
    LAST_RESULTS = res
    out = np.empty((B, S, D), dtype=np.float32)
    for b in range(B):
        acc = (res.results[2 * b]["outT"].astype(np.float32)
               + res.results[2 * b + 1]["outT"].astype(np.float32))
        out[b] = acc.T + host_bias[None, :]
    return out


# revision 130
# speedup vs baseline: 1.0010x; 1.0010x over previous
"""Bark-style causal self-attention on 8 Trainium2 NeuronCores.

Problem (hardcoded): B=4, S=1024, D=1024, H=16, hd=64, fp32 I/O.

Sharding: 8 cores = 4 batches x 2 head-groups (8 heads each).

v2: single fully-interleaved emission stream tuned against the
instruction-cost timeline model:
  - qk^T projection: 4 m-tiles swept k-major at boot (PE consumption rate
    matches the DMA arrival rate of the wqk/hsT chunks), remaining m-tiles
    interleaved into the attention pairs.
  - scores transposed as in v1 (pair-packed, 256-wide query chunks so a
    score tile fits one PSUM bank), exp on Activation, causal mask on DVE.
  - PV with p^T *stationary* and V moving (65 rows per matmul instead of
    ~128-512): ctx comes out natural [q, hd] with the softmax denominator
    in column 64; normalization is then a per-partition scalar multiply.
  - ctx^T recovered with PE transpose instructions (free Ldweights +
    128-row transposes), unloaded PSUM->SBUF on GpSimd.
  - out^T projection per (d, n) group with PSUM accumulation over the 4
    head pairs, n=0 half interleaved into pair 3, biases on GpSimd,
    output stored bf16 (host combines the two cores of a batch in fp32).
"""

from contextlib import ExitStack

import numpy as np
import ml_dtypes

import concourse.bass as bass
import concourse.tile as tile
import concourse.mybir as mybir
from concourse.bass_utils import run_bass_kernel_spmd
from concourse.vector_clock import ScopedClock


# --------------------------------------------------------------------------
# Workaround for the walrus build in this container, which accepts at most
# ONE sync-wait command per instruction (two on EventSemaphore).  Stock Tile
# emits instructions with several waits; we legalize the program after
# TileContext exit (see v1 for details).
# --------------------------------------------------------------------------

def _patched_drain_and_barrier(self, tick_clock, wait_clock):
    drain_inst = self.nc.sync.drain()
    wait_clock.add_sem_waits(
        drain_inst.ins, ScopedClock({None: tick_clock.global_clock})
    )
    si = drain_inst.ins.sync_info
    waits = list(si.on_wait or []) if si is not None else []
    if len(waits) > 1:
        si.on_wait = [waits[0]]
        for w in waits[1:]:
            extra = self.nc.sync.drain()
            esi = extra.ins.sync_info
            if esi is None:
                extra.ins.sync_info = mybir.SyncInfo(on_wait=[w], on_update=[])
            else:
                esi.on_wait = [w]

    self.nc.all_engine_barrier()
    assert self.sems is not None
    popped = self.nc._tile_sem_poison_stack.pop()
    assert popped is self._sem_poison
    self.nc.clear_and_free_semaphores(list(self.sems.allocated().values()))
    self.nc.all_engine_barrier()


tile.TileContext._drain_and_barrier = _patched_drain_and_barrier


def _legalize_waits_json(raw: bytes) -> bytes:
    """Split multi-wait instructions by inserting single-wait NoOp carriers
    immediately before them on the same engine."""
    import orjson

    j = orjson.loads(raw)
    for f in j["functions"]:
        for b in f["blocks"]:
            out = []
            for inst in b["instructions"]:
                si = inst.get("sync_info") or {}
                waits = si.get("on_wait") or []
                cap = 2 if inst.get("opcode") == "EventSemaphore" else 1
                if len(waits) > cap:
                    excess, keep = waits[:-cap], waits[-cap:]
                    for k, w in enumerate(excess):
                        out.append({
                            "debug": inst.get("debug", 0),
                            "engine": inst["engine"],
                            "ins": [],
                            "name": f"{inst['name']}-lw{k}",
                            "opcode": "NoOp",
                            "outs": [],
                            "sync_info": {"on_wait": [w]},
                        })
                    si["on_wait"] = keep
                    inst["sync_info"] = si
                out.append(inst)
            b["instructions"] = out
    return orjson.dumps(j)


BF16 = mybir.dt.bfloat16
F32 = mybir.dt.float32
NPBF16 = ml_dtypes.bfloat16

B, S, D, H, HD = 4, 1024, 1024, 16, 64
NCORES = 8
HPC = 8          # heads per core
PAIRS = 4        # head pairs per core
KCH = 8          # 128-row chunks of the D contraction
SCALE = 1.0 / np.sqrt(HD)
SCH = 256        # score chunk width (query dim); one PSUM bank per sT tile

TRACE = False
LAST_RESULTS = None

_CACHE = {}
DEBUG_DUMP = False


def _chunks(lo, hi, step):
    out = []
    while lo < hi:
        nxt = min(hi, (lo // step + 1) * step)
        out.append((lo, nxt))
        lo = nxt
    return out


def _emit(tc, io, ctx):
    nc = tc.nc
    hsT, wqk, qkb, wv, wout, tri, outT = (
        io["hsT"], io["wqk"], io["qkb"], io["wv"], io["wout"],
        io["tri"], io["outT"],
    )
    Exp = mybir.ActivationFunctionType.Exp

    persist = ctx.enter_context(tc.tile_pool(name="persist", bufs=1))

    def ptile(name, shape, dtype=BF16):
        return persist.tile(shape, dtype, name=name, tag=name)

    # ---- persistent SBUF tensors ----------------------------------------
    qkb_sb = ptile("qkb", [128, 8], F32)
    wqk_sb = [ptile(f"wqk{k}", [128, 1024]) for k in range(KCH)]
    hsT_sb = [ptile(f"hsT{k}", [128, S]) for k in range(KCH)]
    tri_sb = ptile("tri", [128, 128])
    wv_sb = [ptile(f"wv{k}", [128, 512]) for k in range(KCH)]
    wout_sb = [ptile(f"wout{p}", [128, 1024]) for p in range(PAIRS)]

    qkT_sb = [ptile(f"qkT{m}", [128, S]) for m in range(8)]
    v_sb = [ptile(f"v{s}", [128, HPC, 65]) for s in range(8)]
    ctxT_sb = [ptile(f"ctxT{p}", [128, S]) for p in range(PAIRS)]
    ctn_sb = [ptile(f"ctn{p}", [128, 8, 2, HD]) for p in range(PAIRS)]

    # ---- DMA loads (SP queue, in order of first use) --------------------
    # wqk[0][:, 0:768] covers the m in {0, 1, 4, 5} column slices the boot
    # sweep needs; the first matmul can start after just 2 transfers.
    nc.sync.dma_start(out=wqk_sb[0][:, 0:768], in_=wqk[0:128, 0:768])
    nc.sync.dma_start(out=hsT_sb[0][:, 0:512], in_=hsT[0:128, 0:512])
    nc.sync.dma_start(out=hsT_sb[0][:, 512:1024], in_=hsT[0:128, 512:1024])
    for k in range(1, KCH):
        r = slice(k * 128, (k + 1) * 128)
        nc.sync.dma_start(out=wqk_sb[k][:, 0:768], in_=wqk[r, 0:768])
        nc.sync.dma_start(out=hsT_sb[k][:, :], in_=hsT[r, :])
    nc.sync.dma_start(out=qkb_sb[:, :], in_=qkb[:, :])
    nc.sync.dma_start(out=tri_sb[:, :], in_=tri[:, :])
    for k in range(KCH):
        nc.sync.dma_start(out=wv_sb[k][:, :], in_=wv[k * 128:(k + 1) * 128, :])
    for k in range(KCH):   # m in {6, 7} slices, first used in pair 1
        nc.sync.dma_start(out=wqk_sb[k][:, 768:1024],
                          in_=wqk[k * 128:(k + 1) * 128, 768:1024])
    for p in range(PAIRS):
        nc.sync.dma_start(out=wout_sb[p][:, :],
                          in_=wout[p * 128:(p + 1) * 128, :])

    # ---- pools ----------------------------------------------------------
    # PSUM budget: boot(6) + pj(2) = 8 early; pj(2)+sT(2)+ctx(3)+T(1) = 8
    # once boot closes.
    pj_pool = ctx.enter_context(tc.tile_pool(name="pj", bufs=2, space="PSUM"))
    # SBUF working pools
    pt_pool = ctx.enter_context(tc.tile_pool(name="pt", bufs=17))
    rc_pool = ctx.enter_context(tc.tile_pool(name="rc", bufs=2))
    osb_pool = ctx.enter_context(tc.tile_pool(name="osb", bufs=8))

    # ---------------------------------------------------------------------
    # emission helpers
    # ---------------------------------------------------------------------
    def qk_bias(m, ps_n, act_n0=False):
        """PSUM -> SBUF with per-feature bias; the n=1 half (and optionally
        the n=0 half) unloads via an Act copy (+ in-place DVE add) so the
        boot handoff isn't serialized on DVE alone."""
        if act_n0:
            nc.scalar.copy(qkT_sb[m][:, 0:512], ps_n[0][:, :])
            nc.vector.tensor_scalar_add(
                qkT_sb[m][:, 0:512], qkT_sb[m][:, 0:512],
                qkb_sb[:, m:m + 1])
        else:
            nc.vector.tensor_scalar_add(
                qkT_sb[m][:, 0:512], ps_n[0][:, :], qkb_sb[:, m:m + 1])
        nc.scalar.copy(qkT_sb[m][:, 512:1024], ps_n[1][:, :])
        nc.vector.tensor_scalar_add(
            qkT_sb[m][:, 512:1024], qkT_sb[m][:, 512:1024],
            qkb_sb[:, m:m + 1])

    def proj_sweep_pieces(m):
        """k-sweep for one qk m-tile as 9 small pieces (for interleaving)."""
        ps = [None, None]

        def piece(k):
            if k == 0:
                for n in range(2):
                    ps[n] = pj_pool.tile([128, 512], F32,
                                         name=f"pj{m}_{n}", tag="pj")
            for n in range(2):
                nc.tensor.matmul(
                    ps[n][:, :],
                    lhsT=wqk_sb[k][:, m * 128:(m + 1) * 128],
                    rhs=hsT_sb[k][:, n * 512:(n + 1) * 512],
                    start=(k == 0), stop=(k == KCH - 1))

        for k in range(KCH):
            yield lambda k=k: piece(k)
        yield lambda: qk_bias(m, ps)

    def v_proj(s):
        """V projection chunk s: psum -> v_sb[s] (copy on DVE) + ones col."""
        ps = pj_pool.tile([128, 512], F32, name=f"vps{s}", tag="pj")
        for k in range(KCH):
            nc.tensor.matmul(
                ps[:, :],
                lhsT=hsT_sb[k][:, s * 128:(s + 1) * 128],
                rhs=wv_sb[k][:, :],
                start=(k == 0), stop=(k == KCH - 1))
        nc.vector.tensor_copy(v_sb[s][:, :, 0:64],
                              ps.rearrange("p (h c) -> p h c", c=64))
        nc.vector.memset(v_sb[s][:, :, 64:65], 1.0)

    # per-pair attention state
    def scores(p, kb):
        """Pair-packed transposed score chunks + exp + mask (v1 pattern:
        each matmul output fills its own PSUM bank)."""
        q0 = kb * 128
        for (c0, c1) in _chunks(0, S - q0, 512):
            wc = c1 - c0
            sT = sT_pool.tile([128, 2, 512], F32, name=f"sT{p}_{kb}_{c0}",
                              tag="sT")
            for t in range(2):
                nc.tensor.matmul(
                    sT[:, t, 0:wc],
                    lhsT=qkT_sb[4 + p][64 * t:64 * t + 64, q0:q0 + 128],
                    rhs=qkT_sb[p][64 * t:64 * t + 64, q0 + c0:q0 + c1],
                    start=True, stop=True,
                    tile_position=(64 * t, 0))
            pt = pt_pool.tile([128, 2, 512], BF16, name=f"pT{p}_{kb}_{c0}",
                              tag="pT")
            nc.scalar.activation(pt[:, :, 0:wc], sT[:, :, 0:wc], Exp,
                                 scale=SCALE)
            if c0 == 0:
                # causal mask on the diagonal 128x128 block, both heads
                pm = pt[:, :, 0:128]
                tri3 = tri_sb.rearrange("p (o c) -> p o c", o=1)
                tri_b, _ = bass.broadcast_tensor_aps(tri3, pm)
                nc.vector.tensor_mul(pm, pm, tri_b)
            yield pt, c0, c1

    def pv_qb(p, qb, pts):
        """p-stationary PV for one query block, both heads: ctx comes out
        natural [q, 65] (65 moving rows per matmul), the softmax denominator
        is per-partition (cheap normalize), and ctx^T is recovered with a
        hardware DMA transpose.  One accumulation group per PSUM bank."""
        for t in range(2):
            ct = ctx_pool.tile([128, 65], F32, name=f"cx{p}_{qb}_{t}",
                               tag="ctx")
            for kb in range(qb + 1):
                off = (qb - kb) * 128
                pt, c0, c1 = pts[kb][off // 512]
                sl = off - c0
                nc.tensor.matmul(
                    ct[:, :],
                    lhsT=pt[:, t, sl:sl + 128],
                    rhs=v_sb[kb][:, 2 * p + t, :],
                    start=(kb == 0), stop=(kb == qb))
            rc = rc_pool.tile([128, 1], F32, name=f"rc{p}{qb}{t}", tag="rc")
            nc.vector.reciprocal(rc[:, :], ct[:, 64:65])
            nc.vector.tensor_scalar_mul(ctn_sb[p][:, qb, t, :],
                                        ct[:, 0:64], rc[:, 0:1])
        nc.sync.dma_start_transpose(
            ctxT_sb[p][:, qb * 128:(qb + 1) * 128], ctn_sb[p][:, qb, :, :])

    ph4_state = {}

    def ph4_mm(ps, d, n, p, cols=None):
        c0, c1 = cols if cols is not None else (n * 512, (n + 1) * 512)
        nc.tensor.matmul(
            ps[:, c0 - n * 512:c1 - n * 512],
            lhsT=wout_sb[p][:, d * 128:(d + 1) * 128],
            rhs=ctxT_sb[p][:, c0:c1],
            start=(p == 0), stop=(p == PAIRS - 1),
            skip_group_check=cols is not None)

    def phase4_head(d, n, pool=None):
        """Pairs 0..2 of out^T tile (d, n) (not gated on pair 3)."""
        pool = pool if pool is not None else pj_pool
        ps = pool.tile([128, 512], F32, name=f"o{d}_{n}", tag="pj")
        ph4_state[(d, n)] = ps
        for p in range(3):
            ph4_mm(ps, d, n, p)

    osb_tiles = {}

    def phase4_tail(d, n, on_dve=False):
        """Pair-3 matmul + bf16 unload (the output bias is added on the
        host).  Both n-halves collect into one osb tile; a single combined
        DMA per d fires with the n=1 half (8 stores instead of 16)."""
        ps = ph4_state.pop((d, n))
        ph4_mm(ps, d, n, 3)
        if d not in osb_tiles:
            osb_tiles[d] = osb_pool.tile([128, 1024], BF16, name=f"ob{d}",
                                         tag="osb")
        osb = osb_tiles[d]
        if on_dve:
            nc.vector.tensor_copy(osb[:, n * 512:(n + 1) * 512], ps[:, :])
        else:
            nc.scalar.copy(osb[:, n * 512:(n + 1) * 512], ps[:, :])
        # d 5-7 finish last: fire their n=0 halves early (HWDGE is idle
        # then) so only half-sized transfers remain on the critical tail
        if d >= 5:
            nc.sync.dma_start(
                out=outT[d * 128:(d + 1) * 128, n * 512:(n + 1) * 512],
                in_=osb[:, n * 512:(n + 1) * 512])
        elif n == 1:
            nc.sync.dma_start(out=outT[d * 128:(d + 1) * 128, :],
                              in_=osb[:, :])

    def phase4_group(d, n, on_dve=False):
        phase4_head(d, n)
        phase4_tail(d, n, on_dve=on_dve)

    # ---------------------------------------------------------------------
    # boot: m-tiles {0, 4, 1, 5} swept k-major, paced by the input DMAs
    # ---------------------------------------------------------------------
    boot_pool = tc.alloc_tile_pool(name="boot", bufs=1, space="PSUM")
    boot_ms = [0, 4, 1]      # tiles in boot pool (6 banks)
    pjm = 5                  # fourth tile in pj pool (2 banks)
    boot_ps = {m: [boot_pool.tile([128, 512], F32, name=f"bt{m}_{n}",
                                  tag=f"bt{m}_{n}")
                   for n in range(2)] for m in boot_ms}
    pj_ps = {pjm: [pj_pool.tile([128, 512], F32, name=f"pj5_{n}", tag="pj")
                   for n in range(2)]}
    for k in range(KCH):
        for n in range(2):
            for m in boot_ms + [pjm]:
                ps = boot_ps[m][n] if m in boot_ps else pj_ps[m][n]
                nc.tensor.matmul(
                    ps[:, :],
                    lhsT=wqk_sb[k][:, m * 128:(m + 1) * 128],
                    rhs=hsT_sb[k][:, n * 512:(n + 1) * 512],
                    start=(k == 0), stop=(k == KCH - 1))
    # bias order: m0/m4 unblock the pair-0 scores, m1 completes the boot
    # pool's readers (releases its banks to the attention pools), m5 frees
    # the two pj slots the V projection uses.
    qk_bias(0, boot_ps[0])
    qk_bias(4, boot_ps[4])
    qk_bias(1, boot_ps[1], act_n0=True)
    qk_bias(pjm, pj_ps[pjm])
    boot_pool.release()

    # attention pools (open after boot closes): ctx 2 + sT 2x2 + pj 2 = 8.
    # ctx is allocated first so sT (stack top) can be released right after
    # the last scores, freeing banks for pair 3's second ctx pool.
    ctx_pool = tc.alloc_tile_pool(name="ctxp", bufs=2, space="PSUM")
    sT_pool = tc.alloc_tile_pool(name="sT", bufs=2, space="PSUM")

    # ---------------------------------------------------------------------
    # attention pairs with interleaved projection / phase-4 work
    # ---------------------------------------------------------------------
    # Filler PE work queues, one per pair, consumed between the score and
    # PV blocks of each key block (that window is where PE would otherwise
    # stall on the exp -> mask chain).
    fillers = {
        0: [],                                  # pair 0 is filled by V proj
        1: list(proj_sweep_pieces(2)) + list(proj_sweep_pieces(6)),
        2: list(proj_sweep_pieces(3)) + list(proj_sweep_pieces(7)),
        # pair 3: pre-stage the first two phase-4 heads (pairs 0-2 only,
        # not gated on pair 3's ctx^T).
        3: [lambda: phase4_head(0, 0), lambda: phase4_head(1, 0)],
    }

    all_pts = {}

    def emit_scores(p, kb_lo=0):
        """Score/exp stream for a pair, with that pair's filler pieces."""
        fq = fillers[p]
        npiece = ([3, 3, 3, 2, 2, 2, 2, 1] if p != 3
                  else [0, 0, 1, 1] + [0] * 4)
        all_pts.setdefault(p, {})
        for kb in range(kb_lo, KCH):
            all_pts[p][kb] = list(scores(p, kb))
            if p == 0:
                v_proj(kb)
            else:
                for _ in range(npiece[kb]):
                    if fq:
                        fq.pop(0)()
        while fq:
            fq.pop(0)()

    for p in range(3):
        emit_scores(p, kb_lo=(0 if p == 0 else 1))
        for qb in range(KCH):
            pv_qb(p, qb, all_pts[p])
            if qb == 5:
                # pre-emit the next pair's first key block so Act starts
                # its exp stream before this pair's PV drains
                all_pts[p + 1] = {0: list(scores(p + 1, 0))}
        del all_pts[p]

    # pair 3: PV query blocks interleave between its two score batches
    # (kb 0 was pre-emitted during pair 2's PV)
    fq3 = fillers[3]
    for kb in range(1, 4):
        all_pts[3][kb] = list(scores(3, kb))
        if kb >= 1 and fq3:
            fq3.pop(0)()
    for qb in range(4):
        pv_qb(3, qb, all_pts[3])
    for kb in range(4, KCH):
        all_pts[3][kb] = list(scores(3, kb))
    # sT's 4 banks free here and ctx only needs 2: the 4-slot phase-4 pool
    # opens now so four more (ungated) heads fill pair-3's exp-cadence
    # stalls without losing end-stagger depth
    sT_pool.release()
    ph4b_pool = tc.alloc_tile_pool(name="ph4b", bufs=4, space="PSUM")
    order = ([(d, 0) for d in range(4)]
             + [(0, 1), (4, 0), (1, 1), (5, 0), (2, 1), (6, 0), (3, 1),
                (7, 0), (4, 1), (5, 1), (6, 1), (7, 1)])
    for qb in range(4, KCH):
        pv_qb(3, qb, all_pts[3])
        if qb >= 4 and qb - 4 < 4:
            phase4_head(*order[2 + (qb - 4)], pool=ph4b_pool)

    # ---------------------------------------------------------------------
    # phase 4: staggered (d, n) tails; each new head takes the slot its
    # just-emitted tail freed (same pool), keeping the in-order PE stream
    # deadlock-free.  n=0 tails first (only need ctx^T columns 0:512).
    # ---------------------------------------------------------------------
    head_pool = {(0, 0): pj_pool, (1, 0): pj_pool,
                 (2, 0): ph4b_pool, (3, 0): ph4b_pool,
                 (0, 1): ph4b_pool, (4, 0): ph4b_pool}
    for i, (d, n) in enumerate(order):
        phase4_tail(d, n, on_dve=i % 2 == 1)
        if i + 6 < len(order):
            head_pool[order[i + 6]] = head_pool[(d, n)]
            phase4_head(*order[i + 6], pool=head_pool[order[i + 6]])
    ph4b_pool.release()
    ctx_pool.release()

    if DEBUG_DUMP:
        for m in range(8):
            nc.sync.dma_start(out=io["dbg_qkT"][m * 128:(m + 1) * 128, :],
                              in_=qkT_sb[m][:, :])
        for s in range(8):
            nc.sync.dma_start(
                out=io["dbg_v"][s * 128:(s + 1) * 128, :],
                in_=v_sb[s].rearrange("p h c -> p (h c)"))
        for p in range(PAIRS):
            nc.sync.dma_start(out=io["dbg_ctxT"][p * 128:(p + 1) * 128, :],
                              in_=ctxT_sb[p][:, :])


def _build():
    nc = bass.Bass("TRN2", target_bir_lowering=False, debug=False,
                   num_devices=NCORES)
    io = {
        "hsT": nc.dram_tensor("hsT", [1024, S], BF16,
                              kind="ExternalInput").ap(),
        "wqk": nc.dram_tensor("wqk", [1024, 1024], BF16,
                              kind="ExternalInput").ap(),
        "qkb": nc.dram_tensor("qkb", [128, 8], F32,
                              kind="ExternalInput").ap(),
        "wv": nc.dram_tensor("wv", [1024, 512], BF16,
                             kind="ExternalInput").ap(),
        "wout": nc.dram_tensor("wout", [512, 1024], BF16,
                               kind="ExternalInput").ap(),
        "tri": nc.dram_tensor("tri", [128, 128], BF16,
                              kind="ExternalInput").ap(),
        "outT": nc.dram_tensor("outT", [1024, S], BF16,
                               kind="ExternalOutput").ap(),
    }
    if DEBUG_DUMP:
        io["dbg_qkT"] = nc.dram_tensor("dbg_qkT", [1024, S], BF16,
                                       kind="ExternalOutput").ap()
        io["dbg_v"] = nc.dram_tensor("dbg_v", [1024, HPC * 65], BF16,
                                     kind="ExternalOutput").ap()
        io["dbg_ctxT"] = nc.dram_tensor("dbg_ctxT", [512, S], BF16,
                                        kind="ExternalOutput").ap()
    with tile.TileContext(nc) as tc:
        with ExitStack() as ctx:
            _emit(tc, io, ctx)
    fixed = _legalize_waits_json(nc.to_json_bytes())
    nc.to_json_bytes = (lambda fixed=fixed: fixed)
    return nc


def _get_nc():
    if "nc" not in _CACHE:
        _CACHE["nc"] = _build()
    return _CACHE["nc"]


def _prep_inputs(hidden_states, att_w, att_b, out_w, out_b):
    """Build the 8 per-core input maps (host-side shard/layout prep)."""
    hs = np.asarray(hidden_states, dtype=np.float32)
    att_w = np.asarray(att_w, dtype=np.float32)
    att_b = np.asarray(att_b, dtype=np.float32)
    out_w = np.asarray(out_w, dtype=np.float32)
    out_b = np.asarray(out_b, dtype=np.float32)

    tri = np.triu(np.ones((128, 128), dtype=np.float32)).astype(NPBF16)

    hsT_all = [np.ascontiguousarray(hs[b].T.astype(NPBF16))
               for b in range(B)]
    per_hg = []
    for hg in range(2):
        lo, hi = hg * 512, (hg + 1) * 512
        wqk = np.ascontiguousarray(
            np.concatenate([att_w[:, lo:hi], att_w[:, D + lo:D + hi]],
                           axis=1).astype(NPBF16))
        qkb = np.concatenate([att_b[lo:hi], att_b[D + lo:D + hi]])
        qkb = np.ascontiguousarray(qkb.reshape(8, 128).T).astype(np.float32)
        wv = np.ascontiguousarray(
            att_w[:, 2 * D + lo:2 * D + hi].astype(NPBF16))
        wout = np.ascontiguousarray(out_w[lo:hi, :].astype(NPBF16))
        per_hg.append((wqk, qkb, wv, wout))
    # Output bias applied on the host.  The v-bias passes through softmax
    # as a constant (weights sum to 1): ctx = ctx0 + bv, so bv @ w_out is
    # folded in here as well.
    host_bias = out_b + att_b[2 * D:3 * D] @ out_w
    in_maps = []
    for c in range(NCORES):
        b, hg = divmod(c, 2)
        wqk, qkb, wv, wout = per_hg[hg]
        in_maps.append({
            "hsT": hsT_all[b],
            "wqk": wqk,
            "qkb": qkb,
            "wv": wv,
            "wout": wout,
            "tri": tri,
        })
    return in_maps, host_bias


def kernel(hidden_states, att_w, att_b, out_w, out_b):
    global LAST_RESULTS
    in_maps, host_bias = _prep_inputs(hidden_states, att_w, att_b,
                                      out_w, out_b)
    nc = _get_nc()
    trace = TRACE
    if trace:
        try:
            from antenv.axon_hooks import get_axon_ntff_profile_hook  # noqa
        except ImportError:
            trace = False
    res = run_bass_kernel_spmd(nc, in_maps, core_ids=list(range(NCORES)),
                               trace=trace)
    LAST_RESULTS = res
    out = np.empty((B, S, D), dtype=np.float32)
    for b in range(B):
        acc = (res.results[2 * b]["outT"].astype(np.float32)
               + res.results[2 * b + 1]["outT"].astype(np.float32))
        out[b] = acc.T + host_bias[None, :]
    return out
